# revision 35
# baseline (speedup 1.0000x reference)
"""Trainium2 Bass kernel for NeuromodulatedAttentionWithElectrodes.

Math simplification (verified to ~1e-6 rel err vs the jax reference):
the "mod" term (dopamine/serotonin) is a per-row constant added to the
score matrix. Row-std (any ddof) is shift-invariant and softmax is
shift-invariant, so the entire dopamine/serotonin/entropy/first-softmax
pipeline cancels out of the final output. What remains, per batch b:

    Qe, Ke, Ve = Q[b]+e, K[b]+e, V[b]+e          (e = emb[electrode_ids])
    Sraw  = Qe @ Ke^T                            [N, N]   (no 1/sqrt(D))
    r[n]  = 1 / rowstd(Sraw, ddof=1)             (folds the 1/32 scale)
    P     = softmax(Sraw * r[:, None], axis=-1)
    AO    = P @ Ve                               [N, D]
    out_b = AO.flat @ fc_w.T + fc_b              [C]

Device-side structure:
  * e-add, Q/K transposition, dtype casts and the fc-weight swizzle all
    happen on the host; the device reads tensors in matmul-native
    layouts (halves HBM traffic, removes all PE transposes of Q/K and
    their PSUM->SBUF copies).
  * attention output is computed transposed (AOT[d,n] = Ve^T @ P2^T)
    using Ve in its natural layout as the stationary operand, so the fc
    contraction over d lands on PSUM partitions and is reduced by
    free-size-1 PE matmuls against a ones-vector (engine-time ~0).
    A PSUM accumulation group must not be interrupted by another start
    in the same bank, so each column's 8 matmuls are consecutive.
  * 1/rowsum softmax normalisation is folded into the tiny per-(t,c)
    fc partials; the final partition reduction runs on GPSIMD
    (partition_all_reduce), keeping the tail off PE/PSUM.
  * the fc stage of batch b-1 is emitted inside batch b's program
    (software pipelining), so its DVE/Pool products and free PE matmuls
    fill the stats/copy dependency stalls of batch b.
  * optional fp8(e4m3) paths: Q/K fp8 with DoubleRow score matmuls;
    V/P^T fp8 with DoubleRow AOT matmuls (2x PE throughput each).

Sharding: data-parallel over batch, 8 batches per core on 8 cores.
"""

import numpy as np

B, N, D, C = 64, 512, 1024, 4
NCORES = 8
BPC = B // NCORES  # batches per core
P = 128            # partitions
NB = N // P        # 4 row blocks (t / mc)
DB = D // P        # 8 d chunks (dc)

FP8_QK = False     # Q/K fp8 + DoubleRow scores (raw; too lossy)
COMP_QK = True     # Q/K as fp8 value+residual pairs; scores via 3 DoubleRow
                   # groups (drops the residual*residual term, ~1e-4 rel)
FP8_AV = False     # V/P^T fp8 + DoubleRow AOT (raw V; too lossy)
COMP_AV = True     # V as fp8 value+residual, P^T raw fp8; AOT via 4
                   # DoubleRow matmuls per d-chunk
POOL_DC = (6, 7)   # fc-product dc chunks routed to GPSIMD (rest on DVE)

_prog_cache = {}


def _build_program():
    from contextlib import ExitStack

    import concourse.bass_isa as bass_isa
    import concourse.mybir as mybir
    import concourse.tile as tile
    from concourse import bacc
    from concourse.bass import ts
    from concourse.masks import make_identity

    f32 = mybir.dt.float32
    bf16 = mybir.dt.bfloat16
    fp8 = mybir.dt.float8e4
    AF = mybir.ActivationFunctionType
    DR = mybir.MatmulPerfMode.DoubleRow
    qk_dt = fp8 if FP8_QK else bf16
    av_dt = fp8 if (FP8_AV or COMP_AV) else bf16

    # Steer the ACT-table-set chooser: our kernel uses Exp+Ln (+Copy/
    # Identity, present in every set). Hide Exp/Ln from every set except
    # the combined one so a single resident table covers the whole kernel.
    from concourse import hw_specs as _hws
    if not getattr(bacc, "_act_tbl_patched", False):
        _orig_gat = _hws.get_activation_tables

        def _patched_gat(arch):
            t = _orig_gat(arch)
            AF_ = mybir.ActivationFunctionType
            for name, fns in t.items():
                if name != "natural_log_exp_and_others":
                    fns.discard(AF_.Exp)
                    fns.discard(AF_.Ln)
            return t

        bacc.get_activation_tables = _patched_gat
        bacc._act_tbl_patched = True

    nc = bacc.Bacc("TRN2", target_bir_lowering=False, debug=False,
                   num_devices=NCORES)
    # layouts (host-prepared):
    #   qt/kt: [b][p][(dc n)] with d = 8*p + dc  (Qe^T reshaped [128,8,N])
    #   v:     [b][p][(t d)]  with m = t*128 + p
    #   wt:    [c][p][(dc n)] with d = dc*128 + p (matches AOT partitions)
    qk_dt = fp8 if COMP_QK else qk_dt
    qt_d = nc.declare_dram_parameter("qt", [BPC, P, DB * N], qk_dt, isOutput=False)
    kt_d = nc.declare_dram_parameter("kt", [BPC, P, DB * N], qk_dt, isOutput=False)
    if COMP_QK:
        qtr_d = nc.declare_dram_parameter("qtr", [BPC, P, DB * N], fp8, isOutput=False)
        ktr_d = nc.declare_dram_parameter("ktr", [BPC, P, DB * N], fp8, isOutput=False)
    v_d = nc.declare_dram_parameter("v", [BPC, P, NB * D], av_dt, isOutput=False)
    if COMP_AV:
        vr_d = nc.declare_dram_parameter("vr", [BPC, P, NB * D], fp8, isOutput=False)
    wt_d = nc.declare_dram_parameter("wt", [C, P, DB * N], bf16, isOutput=False)
    fcb_d = nc.declare_dram_parameter("fcb", [1, C], f32, isOutput=False)
    out_d = nc.declare_dram_parameter("out", [1, BPC * C], f32, isOutput=True)

    with tile.TileContext(nc) as tc, ExitStack() as ctx:
        # --- pools ---
        const_p = ctx.enter_context(tc.tile_pool(name="const", bufs=1))
        qkv_p = ctx.enter_context(tc.tile_pool(name="qkv", bufs=3))
        trp_p = ctx.enter_context(tc.tile_pool(name="trp", bufs=2))
        p_p = ctx.enter_context(tc.tile_pool(name="p", bufs=6))
        aot_p = ctx.enter_context(tc.tile_pool(name="aot", bufs=26))
        pr_p = ctx.enter_context(tc.tile_pool(name="pr", bufs=10))
        st_p = ctx.enter_context(tc.tile_pool(name="st", bufs=16))
        # PSUM: 8 banks total; 2 + 2 + 3 + 1
        psS = ctx.enter_context(tc.tile_pool(name="psS", bufs=3, space="PSUM"))
        psA = ctx.enter_context(tc.tile_pool(name="psA", bufs=2, space="PSUM"))
        psO = ctx.enter_context(tc.tile_pool(name="psO", bufs=2, space="PSUM"))
        psR = ctx.enter_context(tc.tile_pool(name="psR", bufs=1, space="PSUM"))

        # --- residents ---
        ident = const_p.tile([P, P], bf16, tag="ident")
        make_identity(nc, ident[:, :])
        ones = const_p.tile([P, 1], bf16, tag="ones")
        nc.gpsimd.memset(ones[:, :], 1.0)
        fcb_sb = const_p.tile([1, C], f32, tag="fcb")
        nc.sync.dma_start(fcb_sb[:, :], fcb_d[:, :])
        out_all = const_p.tile([1, BPC * C], f32, tag="outall")
        neg3 = const_p.tile([P, 1], f32, tag="neg3")
        nc.gpsimd.memset(neg3[:, :], -3.0)
        wt_sb = []

        def emit_loads(b):
            # halves so the first score matmuls can start before the
            # full tensor has landed (subtile deps track column ranges)
            H = DB * N // 2
            qt = qkv_p.tile([P, DB * N], qk_dt, tag="qt")
            nc.sync.dma_start(qt[:, 0:H], qt_d[b][:, 0:H])
            kt = qkv_p.tile([P, DB * N], qk_dt, tag="kt")
            nc.sync.dma_start(kt[:, 0:H], kt_d[b][:, 0:H])
            nc.sync.dma_start(qt[:, H:], qt_d[b][:, H:])
            nc.sync.dma_start(kt[:, H:], kt_d[b][:, H:])
            qr = kr = None
            if COMP_QK:
                qr = qkv_p.tile([P, DB * N], fp8, tag="qr")
                nc.sync.dma_start(qr[:, 0:H], qtr_d[b][:, 0:H])
                kr = qkv_p.tile([P, DB * N], fp8, tag="kr")
                nc.sync.dma_start(kr[:, 0:H], ktr_d[b][:, 0:H])
                nc.sync.dma_start(qr[:, H:], qtr_d[b][:, H:])
                nc.sync.dma_start(kr[:, H:], ktr_d[b][:, H:])
            vn = qkv_p.tile([P, NB * D], av_dt, tag="vn")
            nc.sync.dma_start(vn[:, :], v_d[b])
            if COMP_AV:
                vr = qkv_p.tile([P, NB * D], fp8, tag="vr")
                nc.sync.dma_start(vr[:, :], vr_d[b])
                return (qt, kt, qr, kr), (vn, vr)
            return (qt, kt, qr, kr), (vn,)

        def emit_scores(qt, kt, qr=None, kr=None):
            """PE scores; DVE bn stats; ACT r + exp. Returns (p_sb, inv)."""
            p_sb, rss = [], []
            for t in range(NB):
                s_ps = psS.tile([P, N], f32, tag="s")
                st6 = st_p.tile([P, 6], f32, tag="st6")
                if COMP_QK:
                    pairs = [(qt, kt), (qt, kr), (qr, kt)]
                    n_mm = len(pairs) * DB // 2
                    i = 0
                    for a_, b_ in pairs:
                        av_ = a_[:, :].rearrange("p (dc n) -> p dc n", dc=DB)
                        bv_ = b_[:, :].rearrange("p (dc n) -> p dc n", dc=DB)
                        for dc in range(0, DB, 2):
                            nc.tensor.matmul(
                                s_ps[:, :],
                                av_[:, dc: dc + 2, t * P: (t + 1) * P],
                                bv_[:, dc: dc + 2, :],
                                start=(i == 0), stop=(i == n_mm - 1),
                                perf_mode=DR)
                            i += 1
                elif FP8_QK:
                    qv = qt[:, :].rearrange("p (dc n) -> p dc n", dc=DB)
                    kv = kt[:, :].rearrange("p (dc n) -> p dc n", dc=DB)
                    for dc in range(0, DB, 2):
                        nc.tensor.matmul(
                            s_ps[:, :],
                            qv[:, dc: dc + 2, t * P: (t + 1) * P],
                            kv[:, dc: dc + 2, :],
                            start=(dc == 0), stop=(dc == DB - 2),
                            perf_mode=DR)
                else:
                    for dc in range(DB):
                        nc.tensor.matmul(
                            s_ps[:, :],
                            qt[:, dc * N + t * P: dc * N + (t + 1) * P],
                            kt[:, ts(dc, N)],
                            start=(dc == 0), stop=(dc == DB - 1))
                nc.vector.bn_stats(st6[:, :], s_ps[:, :])
                mv = st_p.tile([P, 2], f32, tag="mv")
                nc.vector.bn_aggr(mv[:, :], st6[:, :])
                # r = 1/sd = exp(-0.5*ln(var*N/(N-1))); ln+exp+copy share
                # one ACT table set (sqrt does not -> avoids ~2.7us reloads)
                lnv = st_p.tile([P, 1], f32, tag="lnv")
                nc.scalar.activation(lnv[:, :], mv[:, 1:2], AF.Ln,
                                     scale=float(N) / (N - 1.0))
                r = st_p.tile([P, 1], f32, tag="r")
                nc.scalar.activation(r[:, :], lnv[:, :], AF.Exp, scale=-0.5)
                pt_ = p_p.tile([P, N], bf16, tag="p")
                rs = st_p.tile([P, 1], f32, tag="rs")
                # with an fp8 P^T, shift exp by e^-3 so outlier rows
                # (z up to ~6.2) stay below the e4m3 max of 448; the
                # shift cancels exactly through the 1/rowsum factor
                nc.scalar.activation(pt_[:, :], s_ps[:, :], AF.Exp,
                                     scale=r[:, :], accum_out=rs[:, :],
                                     bias=neg3[:, :] if (FP8_AV or COMP_AV) else 0.0)
                p_sb.append(pt_)
                rss.append(rs)
            return p_sb, rss

        def emit_transpose(p_sb):
            """PE transpose P; ACT copies into one [P, NB*N] tile."""
            ptr = trp_p.tile([P, NB * N], av_dt, tag="ptr")
            for mc in range(NB):
                tp = psA.tile([P, N], bf16, tag="tpose")
                for t in range(NB):
                    nc.tensor.matmul(
                        tp[:, ts(t, P)], p_sb[t][:, ts(mc, P)],
                        ident[:, :], is_transpose=True,
                        start=True, stop=True)
                nc.scalar.copy(ptr[:, ts(mc, N)], tp[:, :])
            return ptr

        def emit_aot(vs, ptr):
            """PE: AOT[d,n] = Ve^T @ P2^T (unscaled); ACT copies to bf16."""
            vn = vs[0]
            aot = []
            for dc in range(DB):
                o_ps = psO.tile([P, N], f32, tag="aops")
                if COMP_AV:
                    pv = ptr[:, :].rearrange("p (t n) -> p t n", t=NB)
                    n_mm = len(vs) * NB // 2
                    i = 0
                    for vsrc in vs:
                        vv = vsrc[:, :].rearrange("p (t d) -> p t d", t=NB)
                        for mc in range(0, NB, 2):
                            nc.tensor.matmul(
                                o_ps[:, :],
                                vv[:, mc: mc + 2, dc * P: (dc + 1) * P],
                                pv[:, mc: mc + 2, :],
                                start=(i == 0), stop=(i == n_mm - 1),
                                perf_mode=DR)
                            i += 1
                elif FP8_AV:
                    vv = vn[:, :].rearrange("p (t d) -> p t d", t=NB)
                    pv = ptr[:, :].rearrange("p (t n) -> p t n", t=NB)
                    for mc in range(0, NB, 2):
                        nc.tensor.matmul(
                            o_ps[:, :],
                            vv[:, mc: mc + 2, dc * P: (dc + 1) * P],
                            pv[:, mc: mc + 2, :],
                            start=(mc == 0), stop=(mc == NB - 2),
                            perf_mode=DR)
                else:
                    for mc in range(NB):
                        nc.tensor.matmul(
                            o_ps[:, :],
                            vn[:, mc * D + dc * P: mc * D + (dc + 1) * P],
                            ptr[:, ts(mc, N)],
                            start=(mc == 0), stop=(mc == NB - 1))
                a = aot_p.tile([P, N], bf16, tag="aot")
                nc.scalar.copy(a[:, :], o_ps[:, :])
                aot.append(a)
            return aot

        def emit_fc_products(aot, pool_dc=POOL_DC):
            """DVE/Pool: pr[c][dc] = aot[dc] * wt_c[dc]."""
            prs = []
            for c in range(C):
                row = []
                for dc in range(DB):
                    pr = pr_p.tile([P, N], bf16, tag="prod")
                    eng = nc.gpsimd if dc in pool_dc else nc.vector
                    eng.tensor_mul(pr[:, :], aot[dc][:, :],
                                   wt_sb[c][:, ts(dc, N)])
                    row.append(pr)
                prs.append(row)
            return prs

        def emit_fc_matmuls(prs):
            """PE: free-size-1 matmuls reduce d-partitions into psR."""
            parts_ps = psR.tile([P, NB * C], f32, tag="parts")
            for c in range(C):
                for t in range(NB):
                    col = t * C + c
                    for dc in range(DB):
                        nc.tensor.matmul(
                            parts_ps[:, col: col + 1],
                            prs[c][dc][:, ts(t, P)], ones[:, :],
                            start=(dc == 0), stop=(dc == DB - 1))
            return parts_ps

        def emit_recips(rss):
            inv = []
            for rs in rss:
                iv = st_p.tile([P, 1], f32, tag="iv")
                nc.vector.reciprocal(iv[:, :], rs[:, :])
                inv.append(iv)
            return inv

        def emit_tail(b, parts_ps, inv):
            """ACT scale by 1/rowsum; Pool partition reduce; DVE adds."""
            parts_sb = st_p.tile([P, NB * C], bf16, tag="partsb")
            for t in range(NB):
                nc.scalar.mul(parts_sb[:, ts(t, C)], parts_ps[:, ts(t, C)],
                              inv[t][:, :])
            red = st_p.tile([P, NB * C], f32, tag="red")
            nc.gpsimd.partition_all_reduce(red[:, :], parts_sb[:, :], P,
                                           bass_isa.ReduceOp.add)
            acc = st_p.tile([1, C], f32, tag="acc")
            nc.vector.tensor_add(acc[:, :], red[0:1, 0:C], red[0:1, C:2 * C])
            acc2 = st_p.tile([1, C], f32, tag="acc2")
            nc.vector.tensor_add(acc2[:, :], red[0:1, 2 * C:3 * C],
                                 red[0:1, 3 * C:4 * C])
            acc3 = st_p.tile([1, C], f32, tag="acc3")
            nc.vector.tensor_add(acc3[:, :], acc[:, :], acc2[:, :])
            nc.vector.tensor_add(out_all[:, ts(b, C)], acc3[:, :],
                                 fcb_sb[:, :])

        # --- pipelined main loop (depth 1): the fc stage of batch b-1
        # is emitted inside batch b's program so its DVE/Pool products
        # and free PE matmuls fill batch b's dependency stalls; the
        # one-time wt DMA is ordered after batch-1 loads so it never
        # delays the input prefetch on the serialized DMA engines ---
        tiles = {0: emit_loads(0)}
        pend = None
        for b in range(BPC):
            if b < BPC:
                if b + 1 < BPC:
                    tiles[b + 1] = emit_loads(b + 1)
                if b == 1:
                    for c in range(C):
                        w = const_p.tile([P, DB * N], bf16, tag=f"w{c}")
                        nc.sync.dma_start(w[:, :], wt_d[c])
                        wt_sb.append(w)
                qk, vs = tiles.pop(b)
                p_sb, rss = emit_scores(*qk)
                ptr = emit_transpose(p_sb)
                aot = emit_aot(vs, ptr)
            last = b == BPC - 1
            stages = []
            if pend is not None:
                stages.append(pend)
            if last:
                stages.append((b, aot, rss))
            for pb, paot, prss in stages:
                # in the combined final iteration both fc stages shift
                # product work toward DVE (Pool is 3.4x slower and would
                # gate the epilogue)
                prs = emit_fc_products(paot, pool_dc=(7,) if last else POOL_DC)
                parts_ps = emit_fc_matmuls(prs)
                pinv = emit_recips(prss)
                emit_tail(pb, parts_ps, pinv)
            pend = None if (last or b >= BPC) else (b, aot, rss)

        nc.sync.dma_start(out_d[:, :], out_all[:, :])

    nc.compile()
    return nc


def _make_in_maps(inputs):
    import ml_dtypes

    bf = ml_dtypes.bfloat16
    f8 = ml_dtypes.float8_e4m3fn
    f8 = ml_dtypes.float8_e4m3fn
    qk_np = f8 if FP8_QK else bf
    av_np = f8 if (FP8_AV or COMP_AV) else bf
    Q = np.asarray(inputs["Q"], dtype=np.float32)
    K = np.asarray(inputs["K"], dtype=np.float32)
    V = np.asarray(inputs["V"], dtype=np.float32)
    ids = np.asarray(inputs["electrode_ids"]).astype(np.int64)
    emb = np.asarray(inputs["emb"], dtype=np.float32)
    fc_w = np.asarray(inputs["fc_w"], dtype=np.float32)
    fc_b = np.asarray(inputs["fc_b"], dtype=np.float32)

    e = emb[ids]                                             # [N, D]
    # Q/K transposed to [D, N] then [128, 8, N] (d = 8p + dc)
    qtf = np.ascontiguousarray(
        (Q + e).transpose(0, 2, 1)).reshape(B, P, DB * N)
    ktf = np.ascontiguousarray(
        (K + e).transpose(0, 2, 1)).reshape(B, P, DB * N)
    if COMP_QK:
        qt = qtf.astype(f8)
        kt = ktf.astype(f8)
        qtr = (qtf - qt.astype(np.float32)).astype(f8)
        ktr = (ktf - kt.astype(np.float32)).astype(f8)
    else:
        qt = qtf.astype(qk_np)
        kt = ktf.astype(qk_np)
    # V in [128, 4, D] (m = t*128 + p)
    vf = np.ascontiguousarray(
        (V + e).reshape(B, NB, P, D).transpose(0, 2, 1, 3)
    ).reshape(B, P, NB * D)
    if COMP_AV:
        v = vf.astype(f8)
        vr = (vf - v.astype(np.float32)).astype(f8)
    else:
        v = vf.astype(av_np)
    # fc_w -> [C, D, N] -> [C, 128, 8, N] with d = dc*128 + p
    wt = np.ascontiguousarray(
        fc_w.reshape(C, N, D).transpose(0, 2, 1)
        .reshape(C, DB, P, N).transpose(0, 2, 1, 3)
    ).reshape(C, P, DB * N).astype(bf)
    fcb = np.ascontiguousarray(fc_b.reshape(1, C))

    in_maps = []
    for i in range(NCORES):
        sl = slice(i * BPC, (i + 1) * BPC)
        m = {
            "qt": qt[sl], "kt": kt[sl], "v": v[sl],
            "wt": wt, "fcb": fcb,
        }
        if COMP_QK:
            m["qtr"] = qtr[sl]
            m["ktr"] = ktr[sl]
        if COMP_AV:
            m["vr"] = vr[sl]
        in_maps.append(m)
    return in_maps


def kernel(**inputs):
    from concourse.bass_utils import run_bass_kernel_spmd

    if "prog" not in _prog_cache:
        _prog_cache["prog"] = _build_program()
    nc = _prog_cache["prog"]

    in_maps = _make_in_maps(inputs)
    res = run_bass_kernel_spmd(nc, in_maps, list(range(NCORES)))
    out = np.concatenate(
        [np.asarray(r["out"]).reshape(BPC, C) for r in res.results], axis=0)
    return np.ascontiguousarray(out.astype(np.float32))


# revision 40
# speedup vs baseline: 1.0041x; 1.0041x over previous
"""Trainium2 Bass kernel for NeuromodulatedAttentionWithElectrodes.

Math simplification (verified to ~1e-6 rel err vs the jax reference):
the "mod" term (dopamine/serotonin) is a per-row constant added to the
score matrix. Row-std (any ddof) is shift-invariant and softmax is
shift-invariant, so the entire dopamine/serotonin/entropy/first-softmax
pipeline cancels out of the final output. What remains, per batch b:

    Qe, Ke, Ve = Q[b]+e, K[b]+e, V[b]+e          (e = emb[electrode_ids])
    Sraw  = Qe @ Ke^T                            [N, N]   (no 1/sqrt(D))
    r[n]  = 1 / rowstd(Sraw, ddof=1)             (folds the 1/32 scale)
    P     = softmax(Sraw * r[:, None], axis=-1)
    AO    = P @ Ve                               [N, D]
    out_b = AO.flat @ fc_w.T + fc_b              [C]

Device-side structure:
  * e-add, Q/K transposition, dtype casts and the fc-weight swizzle all
    happen on the host; the device reads tensors in matmul-native
    layouts (halves HBM traffic, removes all PE transposes of Q/K and
    their PSUM->SBUF copies).
  * attention output is computed transposed (AOT[d,n] = Ve^T @ P2^T)
    using Ve in its natural layout as the stationary operand, so the fc
    contraction over d lands on PSUM partitions and is reduced by
    free-size-1 PE matmuls against a ones-vector (engine-time ~0).
    A PSUM accumulation group must not be interrupted by another start
    in the same bank, so each column's 8 matmuls are consecutive.
  * 1/rowsum softmax normalisation is folded into the tiny per-(t,c)
    fc partials; the final partition reduction runs on GPSIMD
    (partition_all_reduce), keeping the tail off PE/PSUM.
  * the fc stage of batch b-1 is emitted inside batch b's program
    (software pipelining), so its DVE/Pool products and free PE matmuls
    fill the stats/copy dependency stalls of batch b.
  * optional fp8(e4m3) paths: Q/K fp8 with DoubleRow score matmuls;
    V/P^T fp8 with DoubleRow AOT matmuls (2x PE throughput each).

Sharding: data-parallel over batch, 8 batches per core on 8 cores.
"""

import numpy as np

B, N, D, C = 64, 512, 1024, 4
NCORES = 8
BPC = B // NCORES  # batches per core
P = 128            # partitions
NB = N // P        # 4 row blocks (t / mc)
DB = D // P        # 8 d chunks (dc)

FP8_QK = False     # Q/K fp8 + DoubleRow scores (raw; too lossy)
COMP_QK = True     # Q/K as fp8 value+residual pairs; scores via 3 DoubleRow
                   # groups (drops the residual*residual term, ~1e-4 rel)
FP8_AV = False     # V/P^T fp8 + DoubleRow AOT (raw V; too lossy)
COMP_AV = True     # V as fp8 value+residual, P^T raw fp8; AOT via 4
                   # DoubleRow matmuls per d-chunk
POOL_DC = (6, 7)   # fc-product dc chunks routed to GPSIMD (rest on DVE)

_prog_cache = {}


def _build_program():
    from contextlib import ExitStack

    import concourse.bass_isa as bass_isa
    import concourse.mybir as mybir
    import concourse.tile as tile
    from concourse import bacc
    from concourse.bass import ts
    from concourse.masks import make_identity

    f32 = mybir.dt.float32
    bf16 = mybir.dt.bfloat16
    fp8 = mybir.dt.float8e4
    AF = mybir.ActivationFunctionType
    DR = mybir.MatmulPerfMode.DoubleRow
    qk_dt = fp8 if FP8_QK else bf16
    av_dt = fp8 if (FP8_AV or COMP_AV) else bf16

    # Steer the ACT-table-set chooser: our kernel uses Exp+Ln (+Copy/
    # Identity, present in every set). Hide Exp/Ln from every set except
    # the combined one so a single resident table covers the whole kernel.
    from concourse import hw_specs as _hws
    if not getattr(bacc, "_act_tbl_patched", False):
        _orig_gat = _hws.get_activation_tables

        def _patched_gat(arch):
            t = _orig_gat(arch)
            AF_ = mybir.ActivationFunctionType
            for name, fns in t.items():
                if name != "natural_log_exp_and_others":
                    fns.discard(AF_.Exp)
                    fns.discard(AF_.Ln)
            return t

        bacc.get_activation_tables = _patched_gat
        bacc._act_tbl_patched = True

    nc = bacc.Bacc("TRN2", target_bir_lowering=False, debug=False,
                   num_devices=NCORES)
    # layouts (host-prepared):
    #   qt/kt: [b][p][(dc n)] with d = 8*p + dc  (Qe^T reshaped [128,8,N])
    #   v:     [b][p][(t d)]  with m = t*128 + p
    #   wt:    [c][p][(dc n)] with d = dc*128 + p (matches AOT partitions)
    qk_dt = fp8 if COMP_QK else qk_dt
    qt_d = nc.declare_dram_parameter("qt", [BPC, P, DB * N], qk_dt, isOutput=False)
    kt_d = nc.declare_dram_parameter("kt", [BPC, P, DB * N], qk_dt, isOutput=False)
    if COMP_QK:
        qtr_d = nc.declare_dram_parameter("qtr", [BPC, P, DB * N], fp8, isOutput=False)
        ktr_d = nc.declare_dram_parameter("ktr", [BPC, P, DB * N], fp8, isOutput=False)
    v_d = nc.declare_dram_parameter("v", [BPC, P, NB * D], av_dt, isOutput=False)
    if COMP_AV:
        vr_d = nc.declare_dram_parameter("vr", [BPC, P, NB * D], fp8, isOutput=False)
    wt_d = nc.declare_dram_parameter("wt", [C, P, DB * N], bf16, isOutput=False)
    fcb_d = nc.declare_dram_parameter("fcb", [1, C], f32, isOutput=False)
    out_d = nc.declare_dram_parameter("out", [1, BPC * C], f32, isOutput=True)

    with tile.TileContext(nc) as tc, ExitStack() as ctx:
        # --- pools ---
        const_p = ctx.enter_context(tc.tile_pool(name="const", bufs=1))
        qkv_p = ctx.enter_context(tc.tile_pool(name="qkv", bufs=3))
        trp_p = ctx.enter_context(tc.tile_pool(name="trp", bufs=2))
        p_p = ctx.enter_context(tc.tile_pool(name="p", bufs=6))
        aot_p = ctx.enter_context(tc.tile_pool(name="aot", bufs=26))
        pr_p = ctx.enter_context(tc.tile_pool(name="pr", bufs=10))
        st_p = ctx.enter_context(tc.tile_pool(name="st", bufs=16))
        # PSUM: 8 banks total; 2 + 2 + 3 + 1
        psS = ctx.enter_context(tc.tile_pool(name="psS", bufs=3, space="PSUM"))
        psA = ctx.enter_context(tc.tile_pool(name="psA", bufs=2, space="PSUM"))
        psO = ctx.enter_context(tc.tile_pool(name="psO", bufs=2, space="PSUM"))
        psR = ctx.enter_context(tc.tile_pool(name="psR", bufs=1, space="PSUM"))

        # --- residents ---
        ident = const_p.tile([P, P], bf16, tag="ident")
        make_identity(nc, ident[:, :])
        ones = const_p.tile([P, 1], bf16, tag="ones")
        nc.gpsimd.memset(ones[:, :], 1.0)
        fcb_sb = const_p.tile([1, C], f32, tag="fcb")
        nc.sync.dma_start(fcb_sb[:, :], fcb_d[:, :])
        out_all = const_p.tile([1, BPC * C], f32, tag="outall")
        neg3 = const_p.tile([P, 1], f32, tag="neg3")
        nc.gpsimd.memset(neg3[:, :], -3.0)
        wt_sb = []

        def emit_loads(b):
            # halves so the first score matmuls can start before the
            # full tensor has landed (subtile deps track column ranges)
            H = DB * N // 2
            qt = qkv_p.tile([P, DB * N], qk_dt, tag="qt")
            nc.sync.dma_start(qt[:, 0:H], qt_d[b][:, 0:H])
            kt = qkv_p.tile([P, DB * N], qk_dt, tag="kt")
            nc.sync.dma_start(kt[:, 0:H], kt_d[b][:, 0:H])
            nc.sync.dma_start(qt[:, H:], qt_d[b][:, H:])
            nc.sync.dma_start(kt[:, H:], kt_d[b][:, H:])
            qr = kr = None
            if COMP_QK:
                qr = qkv_p.tile([P, DB * N], fp8, tag="qr")
                nc.sync.dma_start(qr[:, 0:H], qtr_d[b][:, 0:H])
                kr = qkv_p.tile([P, DB * N], fp8, tag="kr")
                nc.sync.dma_start(kr[:, 0:H], ktr_d[b][:, 0:H])
                nc.sync.dma_start(qr[:, H:], qtr_d[b][:, H:])
                nc.sync.dma_start(kr[:, H:], ktr_d[b][:, H:])
            vn = qkv_p.tile([P, NB * D], av_dt, tag="vn")
            nc.sync.dma_start(vn[:, :], v_d[b])
            if COMP_AV:
                vr = qkv_p.tile([P, NB * D], fp8, tag="vr")
                nc.sync.dma_start(vr[:, :], vr_d[b])
                return (qt, kt, qr, kr), (vn, vr)
            return (qt, kt, qr, kr), (vn,)

        def emit_scores(qt, kt, qr=None, kr=None):
            """PE scores; DVE bn stats; ACT r + exp. Returns (p_sb, inv)."""
            p_sb, rss = [], []
            for t in range(NB):
                s_ps = psS.tile([P, N], f32, tag="s")
                st6 = st_p.tile([P, 6], f32, tag="st6")
                if COMP_QK:
                    pairs = [(qt, kt), (qt, kr), (qr, kt)]
                    n_mm = len(pairs) * DB // 2
                    i = 0
                    for a_, b_ in pairs:
                        av_ = a_[:, :].rearrange("p (dc n) -> p dc n", dc=DB)
                        bv_ = b_[:, :].rearrange("p (dc n) -> p dc n", dc=DB)
                        for dc in range(0, DB, 2):
                            nc.tensor.matmul(
                                s_ps[:, :],
                                av_[:, dc: dc + 2, t * P: (t + 1) * P],
                                bv_[:, dc: dc + 2, :],
                                start=(i == 0), stop=(i == n_mm - 1),
                                perf_mode=DR)
                            i += 1
                elif FP8_QK:
                    qv = qt[:, :].rearrange("p (dc n) -> p dc n", dc=DB)
                    kv = kt[:, :].rearrange("p (dc n) -> p dc n", dc=DB)
                    for dc in range(0, DB, 2):
                        nc.tensor.matmul(
                            s_ps[:, :],
                            qv[:, dc: dc + 2, t * P: (t + 1) * P],
                            kv[:, dc: dc + 2, :],
                            start=(dc == 0), stop=(dc == DB - 2),
                            perf_mode=DR)
                else:
                    for dc in range(DB):
                        nc.tensor.matmul(
                            s_ps[:, :],
                            qt[:, dc * N + t * P: dc * N + (t + 1) * P],
                            kt[:, ts(dc, N)],
                            start=(dc == 0), stop=(dc == DB - 1))
                nc.vector.bn_stats(st6[:, :], s_ps[:, :])
                mv = st_p.tile([P, 2], f32, tag="mv")
                nc.vector.bn_aggr(mv[:, :], st6[:, :])
                # r = 1/sd = exp(-0.5*ln(var*N/(N-1))); ln+exp+copy share
                # one ACT table set (sqrt does not -> avoids ~2.7us reloads)
                lnv = st_p.tile([P, 1], f32, tag="lnv")
                nc.scalar.activation(lnv[:, :], mv[:, 1:2], AF.Ln,
                                     scale=float(N) / (N - 1.0))
                r = st_p.tile([P, 1], f32, tag="r")
                nc.scalar.activation(r[:, :], lnv[:, :], AF.Exp, scale=-0.5)
                pt_ = p_p.tile([P, N], bf16, tag="p")
                rs = st_p.tile([P, 1], f32, tag="rs")
                # with an fp8 P^T, shift exp by e^-3 so outlier rows
                # (z up to ~6.2) stay below the e4m3 max of 448; the
                # shift cancels exactly through the 1/rowsum factor
                nc.scalar.activation(pt_[:, :], s_ps[:, :], AF.Exp,
                                     scale=r[:, :], accum_out=rs[:, :],
                                     bias=neg3[:, :] if (FP8_AV or COMP_AV) else 0.0)
                p_sb.append(pt_)
                rss.append(rs)
            return p_sb, rss

        def emit_transpose(p_sb):
            """PE transpose P; ACT copies into one [P, NB*N] tile."""
            ptr = trp_p.tile([P, NB * N], av_dt, tag="ptr")
            for mc in range(NB):
                tp = psA.tile([P, N], bf16, tag="tpose")
                for t in range(NB):
                    nc.tensor.matmul(
                        tp[:, ts(t, P)], p_sb[t][:, ts(mc, P)],
                        ident[:, :], is_transpose=True,
                        start=True, stop=True)
                nc.scalar.copy(ptr[:, ts(mc, N)], tp[:, :])
            return ptr

        def emit_aot(vs, ptr):
            """PE: AOT[d,n] = Ve^T @ P2^T (unscaled); ACT copies to bf16."""
            vn = vs[0]
            aot = []
            for dc in range(DB):
                o_ps = psO.tile([P, N], f32, tag="aops")
                if COMP_AV:
                    pv = ptr[:, :].rearrange("p (t n) -> p t n", t=NB)
                    n_mm = len(vs) * NB // 2
                    i = 0
                    for vsrc in vs:
                        vv = vsrc[:, :].rearrange("p (t d) -> p t d", t=NB)
                        for mc in range(0, NB, 2):
                            nc.tensor.matmul(
                                o_ps[:, :],
                                vv[:, mc: mc + 2, dc * P: (dc + 1) * P],
                                pv[:, mc: mc + 2, :],
                                start=(i == 0), stop=(i == n_mm - 1),
                                perf_mode=DR)
                            i += 1
                elif FP8_AV:
                    vv = vn[:, :].rearrange("p (t d) -> p t d", t=NB)
                    pv = ptr[:, :].rearrange("p (t n) -> p t n", t=NB)
                    for mc in range(0, NB, 2):
                        nc.tensor.matmul(
                            o_ps[:, :],
                            vv[:, mc: mc + 2, dc * P: (dc + 1) * P],
                            pv[:, mc: mc + 2, :],
                            start=(mc == 0), stop=(mc == NB - 2),
                            perf_mode=DR)
                else:
                    for mc in range(NB):
                        nc.tensor.matmul(
                            o_ps[:, :],
                            vn[:, mc * D + dc * P: mc * D + (dc + 1) * P],
                            ptr[:, ts(mc, N)],
                            start=(mc == 0), stop=(mc == NB - 1))
                a = aot_p.tile([P, N], bf16, tag="aot")
                if dc in (3, 7):
                    nc.vector.tensor_copy(a[:, :], o_ps[:, :])
                else:
                    nc.scalar.copy(a[:, :], o_ps[:, :])
                aot.append(a)
            return aot

        def emit_fc_products(aot, pool_dc=POOL_DC):
            """DVE/Pool: pr[c][dc] = aot[dc] * wt_c[dc]."""
            prs = []
            for c in range(C):
                row = []
                for dc in range(DB):
                    pr = pr_p.tile([P, N], bf16, tag="prod")
                    eng = nc.gpsimd if dc in pool_dc else nc.vector
                    eng.tensor_mul(pr[:, :], aot[dc][:, :],
                                   wt_sb[c][:, ts(dc, N)])
                    row.append(pr)
                prs.append(row)
            return prs

        def emit_fc_matmuls(prs):
            """PE: free-size-1 matmuls reduce d-partitions into psR."""
            parts_ps = psR.tile([P, NB * C], f32, tag="parts")
            for c in range(C):
                for t in range(NB):
                    col = t * C + c
                    for dc in range(DB):
                        nc.tensor.matmul(
                            parts_ps[:, col: col + 1],
                            prs[c][dc][:, ts(t, P)], ones[:, :],
                            start=(dc == 0), stop=(dc == DB - 1))
            return parts_ps

        def emit_recips(rss):
            inv = []
            for rs in rss:
                iv = st_p.tile([P, 1], f32, tag="iv")
                nc.vector.reciprocal(iv[:, :], rs[:, :])
                inv.append(iv)
            return inv

        def emit_tail(b, parts_ps, inv):
            """ACT scale by 1/rowsum; Pool partition reduce; DVE adds."""
            parts_sb = st_p.tile([P, NB * C], bf16, tag="partsb")
            for t in range(NB):
                nc.scalar.mul(parts_sb[:, ts(t, C)], parts_ps[:, ts(t, C)],
                              inv[t][:, :])
            red = st_p.tile([P, NB * C], f32, tag="red")
            nc.gpsimd.partition_all_reduce(red[:, :], parts_sb[:, :], P,
                                           bass_isa.ReduceOp.add)
            acc = st_p.tile([1, C], f32, tag="acc")
            nc.vector.tensor_add(acc[:, :], red[0:1, 0:C], red[0:1, C:2 * C])
            acc2 = st_p.tile([1, C], f32, tag="acc2")
            nc.vector.tensor_add(acc2[:, :], red[0:1, 2 * C:3 * C],
                                 red[0:1, 3 * C:4 * C])
            acc3 = st_p.tile([1, C], f32, tag="acc3")
            nc.vector.tensor_add(acc3[:, :], acc[:, :], acc2[:, :])
            nc.vector.tensor_add(out_all[:, ts(b, C)], acc3[:, :],
                                 fcb_sb[:, :])

        # --- pipelined main loop (depth 1): the fc stage of batch b-1
        # is emitted inside batch b's program so its DVE/Pool products
        # and free PE matmuls fill batch b's dependency stalls; the
        # one-time wt DMA is ordered after batch-1 loads so it never
        # delays the input prefetch on the serialized DMA engines ---
        tiles = {0: emit_loads(0)}
        pend = None
        for b in range(BPC):
            if b < BPC:
                if b + 1 < BPC:
                    tiles[b + 1] = emit_loads(b + 1)
                if b == 1:
                    for c in range(C):
                        w = const_p.tile([P, DB * N], bf16, tag=f"w{c}")
                        nc.sync.dma_start(w[:, :], wt_d[c])
                        wt_sb.append(w)
                qk, vs = tiles.pop(b)
                p_sb, rss = emit_scores(*qk)
                ptr = emit_transpose(p_sb)
                aot = emit_aot(vs, ptr)
            last = b == BPC - 1
            stages = []
            if pend is not None:
                stages.append(pend)
            if last:
                stages.append((b, aot, rss))
            for pb, paot, prss in stages:
                # in the combined final iteration both fc stages shift
                # product work toward DVE (Pool is 3.4x slower and would
                # gate the epilogue)
                prs = emit_fc_products(paot, pool_dc=(7,) if last else POOL_DC)
                parts_ps = emit_fc_matmuls(prs)
                pinv = emit_recips(prss)
                emit_tail(pb, parts_ps, pinv)
            pend = None if (last or b >= BPC) else (b, aot, rss)

        nc.sync.dma_start(out_d[:, :], out_all[:, :])

    nc.compile()
    return nc


def _make_in_maps(inputs):
    import ml_dtypes

    bf = ml_dtypes.bfloat16
    f8 = ml_dtypes.float8_e4m3fn
    f8 = ml_dtypes.float8_e4m3fn
    qk_np = f8 if FP8_QK else bf
    av_np = f8 if (FP8_AV or COMP_AV) else bf
    Q = np.asarray(inputs["Q"], dtype=np.float32)
    K = np.asarray(inputs["K"], dtype=np.float32)
    V = np.asarray(inputs["V"], dtype=np.float32)
    ids = np.asarray(inputs["electrode_ids"]).astype(np.int64)
    emb = np.asarray(inputs["emb"], dtype=np.float32)
    fc_w = np.asarray(inputs["fc_w"], dtype=np.float32)
    fc_b = np.asarray(inputs["fc_b"], dtype=np.float32)

    e = emb[ids]                                             # [N, D]
    # Q/K transposed to [D, N] then [128, 8, N] (d = 8p + dc)
    qtf = np.ascontiguousarray(
        (Q + e).transpose(0, 2, 1)).reshape(B, P, DB * N)
    ktf = np.ascontiguousarray(
        (K + e).transpose(0, 2, 1)).reshape(B, P, DB * N)
    if COMP_QK:
        qt = qtf.astype(f8)
        kt = ktf.astype(f8)
        qtr = (qtf - qt.astype(np.float32)).astype(f8)
        ktr = (ktf - kt.astype(np.float32)).astype(f8)
    else:
        qt = qtf.astype(qk_np)
        kt = ktf.astype(qk_np)
    # V in [128, 4, D] (m = t*128 + p)
    vf = np.ascontiguousarray(
        (V + e).reshape(B, NB, P, D).transpose(0, 2, 1, 3)
    ).reshape(B, P, NB * D)
    if COMP_AV:
        v = vf.astype(f8)
        vr = (vf - v.astype(np.float32)).astype(f8)
    else:
        v = vf.astype(av_np)
    # fc_w -> [C, D, N] -> [C, 128, 8, N] with d = dc*128 + p
    wt = np.ascontiguousarray(
        fc_w.reshape(C, N, D).transpose(0, 2, 1)
        .reshape(C, DB, P, N).transpose(0, 2, 1, 3)
    ).reshape(C, P, DB * N).astype(bf)
    fcb = np.ascontiguousarray(fc_b.reshape(1, C))

    in_maps = []
    for i in range(NCORES):
        sl = slice(i * BPC, (i + 1) * BPC)
        m = {
            "qt": qt[sl], "kt": kt[sl], "v": v[sl],
            "wt": wt, "fcb": fcb,
        }
        if COMP_QK:
            m["qtr"] = qtr[sl]
            m["ktr"] = ktr[sl]
        if COMP_AV:
            m["vr"] = vr[sl]
        in_maps.append(m)
    return in_maps


def kernel(**inputs):
    from concourse.bass_utils import run_bass_kernel_spmd

    if "prog" not in _prog_cache:
        _prog_cache["prog"] = _build_program()
    nc = _prog_cache["prog"]

    in_maps = _make_in_maps(inputs)
    res = run_bass_kernel_spmd(nc, in_maps, list(range(NCORES)))
    out = np.concatenate(
        [np.asarray(r["out"]).reshape(BPC, C) for r in res.results], axis=0)
    return np.ascontiguousarray(out.astype(np.float32))
